# revision 1
# baseline (speedup 1.0000x reference)
"""DCN layer kernel for Trainium2 (raw Bass), 8-core data parallel.

Computes out = x_0 * (x_l @ w) + b[:, 0] + x_l for
x_l, x_0: [65536, 1024] f32, w, b: [1024, 1] f32.

Sharding: batch dim split evenly across 8 NeuronCores; w/b replicated.

Host side interleaves x_l/x_0 row blocks into one tensor and
pre-replicates w across the 128 partitions. Per core, a 3-stage
software pipeline over tiles of K=8 row blocks (8MB loads):
  SP   (HWDGE): load interleaved x tiles into a 2-slot SBUF ring
  DVE         : 4 batched ops per tile over [128, K, 1024]:
                  tmp = x_l * w_bcast        (free-dim stride-0 bcast)
                  s[P,K] = rowsum(tmp)       (innermost-axis reduce)
                  m = x_0 * s_bcast          (stride-0 bcast of s)
                  o = m + x_l                (written over the x_0 half)
  ACT  (HWDGE): store finished o tiles
Raw Bass with standalone wait_ge commands — every instruction carries at
most one semaphore wait (this toolchain's walrus rejects more). Each ring
slot has its own DMA-completion semaphore so at most one DMA is in flight
per semaphore (concurrent 16-way SDMA increments on a shared semaphore
would make thresholds ambiguous). Same-engine DVE RAW needs an explicit
chain semaphore (verified on HW: without it, reads race ahead of writes).
"""

from contextlib import ExitStack

import numpy as np

import concourse.bass as bass
from concourse import mybir
from concourse import bass_utils

P = 128  # SBUF partitions
N_CORES = 8
K = 8  # row blocks per tile (8MB x-tile)
XB = 2  # x ring slots

f32 = mybir.dt.float32
MUL = mybir.AluOpType.mult
ADD = mybir.AluOpType.add


def _build(nb, dim, with_b, repeat=1):
    """Per-core program: nb 128-row blocks of width dim, K blocks per tile."""
    assert nb % K == 0
    nt = nb // K
    nit = nt * repeat  # repeat>1 re-runs the pipeline for wall-clock timing
    nc = bass.Bass("TRN2", target_bir_lowering=False, debug=False,
                   enable_asserts=False)
    xin = nc.dram_tensor("xin", [nb, P, 2, dim], f32, kind="ExternalInput").ap()
    w_rep_d = nc.dram_tensor("w_rep_in", [P, dim], f32, kind="ExternalInput").ap()
    if with_b:
        b_rep_d = nc.dram_tensor("b_rep_in", [P, dim], f32, kind="ExternalInput").ap()
    out = nc.dram_tensor("out", [nb * P, dim], f32, kind="ExternalOutput").ap()

    xin_t = xin.rearrange("(t k) p c d -> t p k c d", k=K)  # [nt, P, K, 2, dim]
    out_t = out.rearrange("(t k p) d -> t p k d", p=P, k=K)  # [nt, P, K, dim]

    n_const = 1 + int(with_b)

    with ExitStack() as ctx:
        e = ctx.enter_context
        xbuf = e(nc.sbuf_tensor([P, XB, K, 2, dim], f32))
        tmp = e(nc.sbuf_tensor([P, K, dim], f32))
        wrep = e(nc.sbuf_tensor([P, dim], f32))
        brep = e(nc.sbuf_tensor([P, dim], f32))
        s = e(nc.sbuf_tensor([P, K], f32))
        const_sem = e(nc.semaphore("const_sem"))
        load_sems = [e(nc.semaphore(f"load_sem{j}")) for j in range(XB)]
        store_sems = [e(nc.semaphore(f"store_sem{j}")) for j in range(XB)]
        dve_sem = e(nc.semaphore("dve_sem"))
        chain_sem = e(nc.semaphore("chain_sem"))
        block = e(nc.Block())

        @block.sync
        def _(sync):
            sync.dma_start(out=wrep[:, :], in_=w_rep_d[:, :]).then_inc(const_sem, 16)
            if with_b:
                sync.dma_start(out=brep[:, :], in_=b_rep_d[:, :]).then_inc(
                    const_sem, 16
                )
            for t in range(nit):
                if t >= XB:
                    # slot free only after its previous store (o lives in the
                    # x_0 half of the slot) fully landed in DRAM
                    sync.wait_ge(store_sems[t % XB], 16 * (t // XB))
                sync.dma_start(
                    out=xbuf[:, t % XB, :, :, :], in_=xin_t[t % nt]
                ).then_inc(load_sems[t % XB], 16)

        @block.vector
        def _(vector):
            cnt = [0]

            def chain(inst):
                inst.then_inc(chain_sem, 1)
                cnt[0] += 1
                vector.wait_ge(chain_sem, cnt[0])
                return inst

            vector.wait_ge(const_sem, 16 * n_const)
            w_b = wrep[:, None, :].broadcast_to([P, K, dim])
            if with_b:
                b_b = brep[:, None, :].broadcast_to([P, K, dim])
            s_b = s[:, :, None].broadcast_to([P, K, dim])
            for t in range(nit):
                sl = t % XB
                vector.wait_ge(load_sems[sl], 16 * (t // XB + 1))
                xl = xbuf[:, sl, :, 0, :]  # [P, K, dim]
                x0 = xbuf[:, sl, :, 1, :]  # [P, K, dim]; overwritten by o
                # tmp = x_l * w
                chain(nc.vector.scalar_tensor_tensor(
                    out=tmp[:, :, :], in0=xl, scalar=1.0, in1=w_b,
                    op0=MUL, op1=MUL,
                ))
                # s[p, k] = sum_d tmp[p, k, d]
                chain(nc.vector.tensor_reduce(
                    s[:, :], tmp[:, :, :], axis=mybir.AxisListType.X, op=ADD
                ))
                # m = x_0 * s  (reuses tmp)
                chain(nc.vector.scalar_tensor_tensor(
                    out=tmp[:, :, :], in0=x0, scalar=1.0, in1=s_b,
                    op0=MUL, op1=MUL,
                ))
                # o = m + x_l (+ b), written over the x_0 half of the slot
                if with_b:
                    chain(nc.vector.scalar_tensor_tensor(
                        out=x0, in0=tmp[:, :, :], scalar=0.0, in1=xl,
                        op0=ADD, op1=ADD,
                    ))
                    last = nc.vector.scalar_tensor_tensor(
                        out=x0, in0=x0, scalar=0.0, in1=b_b, op0=ADD, op1=ADD
                    )
                else:
                    last = nc.vector.scalar_tensor_tensor(
                        out=x0, in0=tmp[:, :, :], scalar=0.0, in1=xl,
                        op0=ADD, op1=ADD,
                    )
                last.then_inc(dve_sem, 1)

        @block.scalar
        def _(scalar):
            for t in range(nit):
                scalar.wait_ge(dve_sem, t + 1)
                scalar.dma_start(
                    out=out_t[t % nt], in_=xbuf[:, t % XB, :, 1, :]
                ).then_inc(store_sems[t % XB], 16)
            # drain: all stores landed before program end
            for j in range(XB):
                n_j = (nit - 1 - j) // XB + 1 if j < nit else 0
                if n_j:
                    scalar.wait_ge(store_sems[j], 16 * n_j)

    return nc


_cache = {}


def _get_module(nb, dim, with_b, repeat=1):
    key = (nb, dim, with_b, repeat)
    if key not in _cache:
        _cache[key] = _build(nb, dim, with_b, repeat)
    return _cache[key]


def make_inputs(x_l, x_0, w, b, n_cores=N_CORES):
    """Host-side shard + interleave. Returns (in_maps, with_b, nb, dim)."""
    rows, dim = x_l.shape
    assert rows % (n_cores * P) == 0
    bl = rows // n_cores
    nb = bl // P
    with_b = bool(np.any(b))
    xin = np.stack([x_l, x_0], axis=1)  # [rows, 2, dim]
    w_rep = np.ascontiguousarray(np.broadcast_to(w.reshape(1, dim), (P, dim)))
    if with_b:
        b_rep = np.ascontiguousarray(np.broadcast_to(b.reshape(1, dim), (P, dim)))
    in_maps = []
    for i in range(n_cores):
        m = {
            "xin": xin[i * bl : (i + 1) * bl].reshape(nb, P, 2, dim),
            "w_rep_in": w_rep,
        }
        if with_b:
            m["b_rep_in"] = b_rep
        in_maps.append(m)
    return in_maps, with_b, nb, dim


def run_sharded(x_l, x_0, w, b, trace=False, repeat=1, **kw):
    in_maps, with_b, nb, dim = make_inputs(x_l, x_0, w, b)
    nc = _get_module(nb, dim, with_b, repeat=repeat)
    res = bass_utils.run_bass_kernel_spmd(
        nc, in_maps, core_ids=list(range(N_CORES)), trace=trace, **kw
    )
    out = np.concatenate([res.results[i]["out"] for i in range(N_CORES)], axis=0)
    return out, res


def kernel(x_l, x_0, w, b):
    out, _ = run_sharded(
        np.asarray(x_l), np.asarray(x_0), np.asarray(w), np.asarray(b)
    )
    return out.astype(np.float32, copy=False)



# revision 3
# speedup vs baseline: 1.8875x; 1.8875x over previous
"""DCN layer kernel for Trainium2 (raw Bass), 8-core data parallel, fp16 I/O.

Computes out = x_0 * (x_l @ w) + b[:, 0] + x_l for
x_l, x_0: [65536, 1024] f32, w, b: [1024, 1] f32.

Sharding: batch dim split evenly across 8 NeuronCores; w/b replicated.

The problem is HBM-bandwidth bound at f32 (96 MB/core) and the DVE was the
measured bottleneck of the f32 version (280 us busy of 351 us total: 4 full
element passes at 1 elem/cycle/lane). Two changes attack both at once:

  * fp16 I/O: the host casts x_l/x_0 to fp16 (and upcasts the result), which
    halves HBM traffic (48 MB/core -> ~134 us DMA floor) and enables the DVE
    16-bit 2x mode. Accuracy budget: fp16 has 2^-11 relative rounding; the
    end-to-end max-normalized error lands ~2e-4, far under the 2e-2 gate.
  * fused DVE ops, 2 passes instead of 4, via scalar_tensor_tensor:
      batch1 (per 128-row block k): tmp_k = x_l_k * w, accum_out -> s[:, k]
        (multiply + row-reduce in ONE instruction; tmp is a write-only sink)
      batch2 (per k): o_k = (x_0_k * s[:, k]) + x_l_k
        (per-partition AP scalar s[:, k]; output overwrites the x_0 half)

Host side interleaves x_l/x_0 row blocks into one tensor and pre-replicates
w across the 128 partitions. Per core, a software pipeline over tiles of
K=8 row blocks (4 MB fp16 loads) with an XB=4 SBUF ring:
  SP   (HWDGE): load interleaved x tiles into the ring
  DVE         : 16 fused ops per tile (8x batch1, 8x batch2)
  ACT  (HWDGE): store finished o tiles (2 MB)
Raw Bass with standalone wait_ge commands - every instruction carries at
most one semaphore wait. Each ring slot has its own DMA-completion
semaphore. Same-engine DVE RAW (batch1 -> batch2 reads s) is protected by
one chain-semaphore wait per tile; batch2 -> store by dve_sem (HW-verified
in the f32 predecessor that unsynchronized same-engine RAW races).
"""

from contextlib import ExitStack

import numpy as np

import concourse.bass as bass
from concourse import mybir
from concourse import bass_utils

P = 128  # SBUF partitions
N_CORES = 8
K = 8  # row blocks per tile
XB = 4  # x ring slots

f16 = mybir.dt.float16
MUL = mybir.AluOpType.mult
ADD = mybir.AluOpType.add


def _build(nb, dim, with_b, repeat=1):
    """Per-core program: nb 128-row blocks of width dim, K blocks per tile."""
    assert nb % K == 0
    nt = nb // K
    nit = nt * repeat  # repeat>1 re-runs the pipeline for wall-clock timing
    nc = bass.Bass("TRN2", target_bir_lowering=False, debug=False,
                   enable_asserts=False)
    xin = nc.dram_tensor("xin", [nb, P, 2, dim], f16, kind="ExternalInput").ap()
    w_rep_d = nc.dram_tensor("w_rep_in", [P, dim], f16, kind="ExternalInput").ap()
    if with_b:
        b_rep_d = nc.dram_tensor("b_rep_in", [P, dim], f16, kind="ExternalInput").ap()
    out = nc.dram_tensor("out", [nb * P, dim], f16, kind="ExternalOutput").ap()

    xin_t = xin.rearrange("(t k) p c d -> t p k c d", k=K)  # [nt, P, K, 2, dim]
    out_t = out.rearrange("(t k p) d -> t p k d", p=P, k=K)  # [nt, P, K, dim]

    n_const = 1 + int(with_b)
    # dve_sem increments per tile: 8 batch2 ops (+8 b-adds when with_b)
    m2 = K * (1 + int(with_b))

    with ExitStack() as ctx:
        e = ctx.enter_context
        xbuf = e(nc.sbuf_tensor([P, XB, K, 2, dim], f16))
        tmp = e(nc.sbuf_tensor([P, K, dim], f16))
        wrep = e(nc.sbuf_tensor([P, dim], f16))
        brep = e(nc.sbuf_tensor([P, dim], f16))
        s = e(nc.sbuf_tensor([P, K], f16))
        const_sem = e(nc.semaphore("const_sem"))
        load_sems = [e(nc.semaphore(f"load_sem{j}")) for j in range(XB)]
        store_sems = [e(nc.semaphore(f"store_sem{j}")) for j in range(XB)]
        dve_sem = e(nc.semaphore("dve_sem"))
        chain_sem = e(nc.semaphore("chain_sem"))
        block = e(nc.Block())

        @block.sync
        def _(sync):
            sync.dma_start(out=wrep[:, :], in_=w_rep_d[:, :]).then_inc(const_sem, 16)
            if with_b:
                sync.dma_start(out=brep[:, :], in_=b_rep_d[:, :]).then_inc(
                    const_sem, 16
                )
            for t in range(nit):
                if t >= XB:
                    # slot free only after its previous store (o lives in the
                    # x_0 half of the slot) fully landed in DRAM
                    sync.wait_ge(store_sems[t % XB], 16 * (t // XB))
                sync.dma_start(
                    out=xbuf[:, t % XB, :, :, :], in_=xin_t[t % nt]
                ).then_inc(load_sems[t % XB], 16)

        @block.vector
        def _(vector):
            vector.wait_ge(const_sem, 16 * n_const)
            for t in range(nit):
                sl = t % XB
                vector.wait_ge(load_sems[sl], 16 * (t // XB + 1))
                # batch1: tmp_k = x_l_k * w, s[:, k] = rowsum(tmp_k)
                for k in range(K):
                    nc.vector.scalar_tensor_tensor(
                        out=tmp[:, k, :],
                        in0=xbuf[:, sl, k, 0, :],
                        scalar=1.0,
                        in1=wrep[:, :],
                        op0=MUL,
                        op1=MUL,
                        accum_out=s[:, k : k + 1],
                    ).then_inc(chain_sem, 1)
                # one drain point: all of batch1's s writes landed
                vector.wait_ge(chain_sem, K * (t + 1))
                # batch2: o_k = (x_0_k * s[:, k]) + x_l_k, over the x_0 half
                for k in range(K):
                    nc.vector.scalar_tensor_tensor(
                        out=xbuf[:, sl, k, 1, :],
                        in0=xbuf[:, sl, k, 1, :],
                        scalar=s[:, k : k + 1],
                        in1=xbuf[:, sl, k, 0, :],
                        op0=MUL,
                        op1=ADD,
                    ).then_inc(dve_sem, 1)
                if with_b:
                    # rare path (b is all-zero in this model); serialize the
                    # RAW on the o_k halves with one extra drain point
                    vector.wait_ge(dve_sem, m2 * t + K)
                    for k in range(K):
                        nc.vector.scalar_tensor_tensor(
                            out=xbuf[:, sl, k, 1, :],
                            in0=xbuf[:, sl, k, 1, :],
                            scalar=0.0,
                            in1=brep[:, :],
                            op0=ADD,
                            op1=ADD,
                        ).then_inc(dve_sem, 1)

        @block.scalar
        def _(scalar):
            for t in range(nit):
                scalar.wait_ge(dve_sem, m2 * (t + 1))
                scalar.dma_start(
                    out=out_t[t % nt], in_=xbuf[:, t % XB, :, 1, :]
                ).then_inc(store_sems[t % XB], 16)
            # drain: all stores landed before program end
            for j in range(XB):
                n_j = (nit - 1 - j) // XB + 1 if j < nit else 0
                if n_j:
                    scalar.wait_ge(store_sems[j], 16 * n_j)

    return nc


_cache = {}


def _get_module(nb, dim, with_b, repeat=1):
    key = (nb, dim, with_b, repeat)
    if key not in _cache:
        _cache[key] = _build(nb, dim, with_b, repeat)
    return _cache[key]


def make_inputs(x_l, x_0, w, b, n_cores=N_CORES):
    """Host-side shard + interleave + fp16 cast. Returns (in_maps, with_b, nb, dim)."""
    rows, dim = x_l.shape
    assert rows % (n_cores * P) == 0
    bl = rows // n_cores
    nb = bl // P
    with_b = bool(np.any(b))
    xin = np.empty((rows, 2, dim), dtype=np.float16)
    xin[:, 0, :] = x_l
    xin[:, 1, :] = x_0
    w_rep = np.ascontiguousarray(
        np.broadcast_to(w.reshape(1, dim), (P, dim)).astype(np.float16)
    )
    if with_b:
        b_rep = np.ascontiguousarray(
            np.broadcast_to(b.reshape(1, dim), (P, dim)).astype(np.float16)
        )
    in_maps = []
    for i in range(n_cores):
        m = {
            "xin": xin[i * bl : (i + 1) * bl].reshape(nb, P, 2, dim),
            "w_rep_in": w_rep,
        }
        if with_b:
            m["b_rep_in"] = b_rep
        in_maps.append(m)
    return in_maps, with_b, nb, dim


def run_sharded(x_l, x_0, w, b, trace=False, repeat=1, **kw):
    in_maps, with_b, nb, dim = make_inputs(x_l, x_0, w, b)
    nc = _get_module(nb, dim, with_b, repeat=repeat)
    res = bass_utils.run_bass_kernel_spmd(
        nc, in_maps, core_ids=list(range(N_CORES)), trace=trace, **kw
    )
    out = np.concatenate([res.results[i]["out"] for i in range(N_CORES)], axis=0)
    return out, res


def kernel(x_l, x_0, w, b):
    out, _ = run_sharded(
        np.asarray(x_l), np.asarray(x_0), np.asarray(w), np.asarray(b)
    )
    return out.astype(np.float32, copy=False)


# revision 6
# speedup vs baseline: 2.0336x; 1.0774x over previous
"""DCN layer kernel for Trainium2 (raw Bass), 8-core data parallel, fp16 I/O.

Computes out = x_0 * (x_l @ w) + b[:, 0] + x_l for
x_l, x_0: [65536, 1024] f32, w, b: [1024, 1] f32.

Sharding: batch dim split evenly across 8 NeuronCores; w/b replicated.

The problem is HBM-bandwidth bound. fp16 I/O (host casts inputs, upcasts the
result; rel-err lands ~7e-4, far under the 2e-2 gate) halves HBM traffic to
48 MB/core -> ~141 us DMA floor. The DVE was the pacer of the first fp16
version (153 us busy: 16 fused scalar_tensor_tensor ops/tile at 1x mode -
STT has no 16-bit 2x micro-op program). v2 splits the work across engines:

  DVE  batch1 (per 128-row block k): tmp_k = x_l_k * w, accum_out -> s[:, k]
         (multiply + row-reduce in ONE 1x STT; tmp is a write-only sink)
       batch3: o = m + x_l as ONE whole-tile tensor_tensor add (2x candidate)
  ACT  batch2 (per k): m_k = Copy(x_0_k * scale), scale = s[:, k] per-partition
       + store DMA issue (HWDGE)
  SP   load DMA issue (HWDGE)

Pipelined one tile deep: DVE does b1(t) then add(t-1); ACT does acts(t) then
store(t-1). s and m are double-buffered on tile parity; the cross-engine
semaphores (s_sem: b1 -> acts, act_sem: acts -> add, add_sem: add -> store)
also fence the parity-buffer reuse. Same-engine WAW on the tmp sink is
benign (never read). Tiles are K=4 row blocks (2 MB fp16 loads, 1 MB
stores) with an XB=6 slot SBUF ring; raw Bass, standalone wait_ge commands,
at most one semaphore wait per instruction.
"""

from contextlib import ExitStack

import numpy as np

import concourse.bass as bass
from concourse import mybir
from concourse import bass_utils

P = 128  # SBUF partitions
N_CORES = 8
K = 4  # row blocks per tile
XB = 6  # x ring slots

f16 = mybir.dt.float16
f32 = mybir.dt.float32
MUL = mybir.AluOpType.mult
ADD = mybir.AluOpType.add
COPY = mybir.ActivationFunctionType.Copy


def _build(nb, dim, with_b, repeat=1):
    """Per-core program: nb 128-row blocks of width dim, K blocks per tile."""
    assert nb % K == 0
    nt = nb // K
    nit = nt * repeat  # repeat>1 re-runs the pipeline for wall-clock timing
    nc = bass.Bass("TRN2", target_bir_lowering=False, debug=False,
                   enable_asserts=False)
    xin = nc.dram_tensor("xin", [nb, P, 2, dim], f16, kind="ExternalInput").ap()
    w_rep_d = nc.dram_tensor("w_rep_in", [P, dim], f16, kind="ExternalInput").ap()
    if with_b:
        b_rep_d = nc.dram_tensor("b_rep_in", [P, dim], f16, kind="ExternalInput").ap()
    out = nc.dram_tensor("out", [nb * P, dim], f16, kind="ExternalOutput").ap()

    xin_t = xin.rearrange("(t k) p c d -> t p k c d", k=K)  # [nt, P, K, 2, dim]
    out_t = out.rearrange("(t k p) d -> t p k d", p=P, k=K)  # [nt, P, K, dim]

    n_const = 1 + int(with_b)

    with ExitStack() as ctx:
        e = ctx.enter_context
        xbuf = e(nc.sbuf_tensor([P, XB, K, 2, dim], f16))
        tmp = e(nc.sbuf_tensor([P, K, dim], f16))
        mbuf = e(nc.sbuf_tensor([P, 2, K, dim], f16))
        wrep = e(nc.sbuf_tensor([P, dim], f16))
        brep = e(nc.sbuf_tensor([P, dim], f16))
        s = e(nc.sbuf_tensor([P, 2, K], f32))  # ACT scale APs must be FP32
        const_sem = e(nc.semaphore("const_sem"))
        load_sems = [e(nc.semaphore(f"load_sem{j}")) for j in range(XB)]
        store_sems = [e(nc.semaphore(f"store_sem{j}")) for j in range(XB)]
        s_sem = e(nc.semaphore("s_sem"))
        act_sem = e(nc.semaphore("act_sem"))
        add_sem = e(nc.semaphore("add_sem"))
        chain_sem = e(nc.semaphore("chain_sem"))
        block = e(nc.Block())

        @block.sync
        def _(sync):
            sync.dma_start(out=wrep[:, :], in_=w_rep_d[:, :]).then_inc(const_sem, 16)
            if with_b:
                sync.dma_start(out=brep[:, :], in_=b_rep_d[:, :]).then_inc(
                    const_sem, 16
                )
            for t in range(nit):
                if t >= XB:
                    # slot free only after its previous store (o lives in the
                    # x_0 half of the slot) fully landed in DRAM
                    sync.wait_ge(store_sems[t % XB], 16 * (t // XB))
                sync.dma_start(
                    out=xbuf[:, t % XB, :, :, :], in_=xin_t[t % nt]
                ).then_inc(load_sems[t % XB], 16)

        def emit_b1(t):
            # batch1: tmp_k = x_l_k * w, s[:, t%2, k] = rowsum(tmp_k)
            sl = t % XB
            for k in range(K):
                nc.vector.scalar_tensor_tensor(
                    out=tmp[:, k, :],
                    in0=xbuf[:, sl, k, 0, :],
                    scalar=1.0,
                    in1=wrep[:, :],
                    op0=MUL,
                    op1=MUL,
                    accum_out=s[:, t % 2, k : k + 1],
                ).then_inc(s_sem, 1)

        def emit_add(vector, t):
            # batch3: o(t) = m(t) + x_l(t), whole tile, over the x_0 half
            sl = t % XB
            vector.wait_ge(act_sem, K * (t + 1))
            inst = nc.vector.tensor_tensor(
                out=xbuf[:, sl, :, 1, :],
                in0=mbuf[:, t % 2, :, :],
                in1=xbuf[:, sl, :, 0, :],
                op=ADD,
            )
            if with_b:
                inst.then_inc(chain_sem, 1)
                vector.wait_ge(chain_sem, t + 1)
                inst = nc.vector.tensor_tensor(
                    out=xbuf[:, sl, :, 1, :],
                    in0=xbuf[:, sl, :, 1, :],
                    in1=brep[:, None, :].broadcast_to([P, K, dim]),
                    op=ADD,
                )
            inst.then_inc(add_sem, 1)

        @block.vector
        def _(vector):
            vector.wait_ge(const_sem, 16 * n_const)
            for t in range(nit):
                vector.wait_ge(load_sems[t % XB], 16 * (t // XB + 1))
                emit_b1(t)
                if t >= 1:
                    emit_add(vector, t - 1)
            emit_add(vector, nit - 1)

        @block.scalar
        def _(scalar):
            def emit_acts(t):
                # batch2: m_k = Copy(x_0_k * s[:, t%2, k])
                sl = t % XB
                scalar.wait_ge(s_sem, K * (t + 1))
                for k in range(K):
                    nc.scalar.activation(
                        out=mbuf[:, t % 2, k, :],
                        in_=xbuf[:, sl, k, 1, :],
                        func=COPY,
                        scale=s[:, t % 2, k : k + 1],
                    ).then_inc(act_sem, 1)

            def emit_store(t):
                scalar.wait_ge(add_sem, t + 1)
                scalar.dma_start(
                    out=out_t[t % nt], in_=xbuf[:, t % XB, :, 1, :]
                ).then_inc(store_sems[t % XB], 16)

            for t in range(nit):
                emit_acts(t)
                if t >= 1:
                    emit_store(t - 1)
            emit_store(nit - 1)
            # drain: all stores landed before program end
            for j in range(XB):
                n_j = (nit - 1 - j) // XB + 1 if j < nit else 0
                if n_j:
                    scalar.wait_ge(store_sems[j], 16 * n_j)

    return nc


_cache = {}


def _get_module(nb, dim, with_b, repeat=1):
    key = (nb, dim, with_b, repeat)
    if key not in _cache:
        _cache[key] = _build(nb, dim, with_b, repeat)
    return _cache[key]


def make_inputs(x_l, x_0, w, b, n_cores=N_CORES):
    """Host-side shard + interleave + fp16 cast. Returns (in_maps, with_b, nb, dim)."""
    rows, dim = x_l.shape
    assert rows % (n_cores * P) == 0
    bl = rows // n_cores
    nb = bl // P
    with_b = bool(np.any(b))
    xin = np.empty((rows, 2, dim), dtype=np.float16)
    xin[:, 0, :] = x_l
    xin[:, 1, :] = x_0
    w_rep = np.ascontiguousarray(
        np.broadcast_to(w.reshape(1, dim), (P, dim)).astype(np.float16)
    )
    if with_b:
        b_rep = np.ascontiguousarray(
            np.broadcast_to(b.reshape(1, dim), (P, dim)).astype(np.float16)
        )
    in_maps = []
    for i in range(n_cores):
        m = {
            "xin": xin[i * bl : (i + 1) * bl].reshape(nb, P, 2, dim),
            "w_rep_in": w_rep,
        }
        if with_b:
            m["b_rep_in"] = b_rep
        in_maps.append(m)
    return in_maps, with_b, nb, dim


def run_sharded(x_l, x_0, w, b, trace=False, repeat=1, **kw):
    in_maps, with_b, nb, dim = make_inputs(x_l, x_0, w, b)
    nc = _get_module(nb, dim, with_b, repeat=repeat)
    res = bass_utils.run_bass_kernel_spmd(
        nc, in_maps, core_ids=list(range(N_CORES)), trace=trace, **kw
    )
    out = np.concatenate([res.results[i]["out"] for i in range(N_CORES)], axis=0)
    return out, res


def kernel(x_l, x_0, w, b):
    out, _ = run_sharded(
        np.asarray(x_l), np.asarray(x_0), np.asarray(w), np.asarray(b)
    )
    return out.astype(np.float32, copy=False)


# revision 7
# speedup vs baseline: 2.6099x; 1.2834x over previous
"""DCN layer kernel for Trainium2 (raw Bass), 8-core data parallel, fp16 I/O.

Computes out = x_0 * (x_l @ w) + b[:, 0] + x_l for
x_l, x_0: [65536, 1024] f32, w, b: [1024, 1] f32.

Sharding: batch dim split evenly across 8 NeuronCores; w/b replicated.

The problem is HBM-bandwidth bound. fp16 I/O (host casts inputs, upcasts the
result; rel-err ~8e-4, far under the 2e-2 gate) halves HBM traffic to
48 MB/core -> ~141 us DMA floor at the ~358 GB/s per-NC HBM limit. Work is
split across engines so no engine paces the DMA streams:

  DVE  batch1 (per 128-row block k): tmp_k = x_l_k * w, accum_out -> s[:, k]
         (multiply + row-reduce in ONE 1x scalar_tensor_tensor; tmp is a
         write-only sink, never read)
       batch3: o = m + x_l as two half-tile tensor_tensor adds (2x fp16 mode,
         split so the tail after the last load is short)
  ACT  batch2 (per k): m_k = Copy(x_0_k * scale), scale = s[:, k] (fp32
         per-partition AP), chained per-k on s_sem so it tracks batch1
       + store DMA issue (HWDGE)
  SP   load DMA issue (HWDGE)

Host pre-interleaves tiles in tile-major layout [nt, P, K, 2, dim] so each
tile load is 128 x 16 KB contiguous descriptors (and each store 128 x 8 KB);
the output is un-transposed on the host. Tiles are K=4 row blocks (2 MB
loads / 1 MB stores) with an XB=8 slot ring - deep enough that the load
queue never stalls on the first stores landing (~40 us in). Pipelined one
tile deep: DVE does b1(t) then adds(t-1); ACT does acts(t) then store(t-1).
s and m are double-buffered on tile parity; cross-engine semaphores
(s_sem: b1 -> acts, act_sem: acts -> adds, add_sem: adds -> store) also
fence parity reuse. Raw Bass, standalone wait_ge, at most one semaphore
wait per instruction (HW-verified: same-engine RAW without a semaphore
races).
"""

from contextlib import ExitStack

import numpy as np

import concourse.bass as bass
from concourse import mybir
from concourse import bass_utils

P = 128  # SBUF partitions
N_CORES = 8
K = 4  # row blocks per tile
XB = 8  # x ring slots
H = 2  # tile-add split factor (tail shortening)

f16 = mybir.dt.float16
f32 = mybir.dt.float32
MUL = mybir.AluOpType.mult
ADD = mybir.AluOpType.add
COPY = mybir.ActivationFunctionType.Copy

assert K % H == 0


def _build(nb, dim, with_b, repeat=1):
    """Per-core program: nb 128-row blocks of width dim, K blocks per tile."""
    assert nb % K == 0
    nt = nb // K
    nit = nt * repeat  # repeat>1 re-runs the pipeline for wall-clock timing
    nc = bass.Bass("TRN2", target_bir_lowering=False, debug=False,
                   enable_asserts=False)
    # tile-major: host lays out so each (t, p) slab is K*2*dim contiguous
    xin = nc.dram_tensor("xin", [nt, P, K, 2, dim], f16, kind="ExternalInput").ap()
    w_rep_d = nc.dram_tensor("w_rep_in", [P, dim], f16, kind="ExternalInput").ap()
    if with_b:
        b_rep_d = nc.dram_tensor("b_rep_in", [P, dim], f16, kind="ExternalInput").ap()
    out = nc.dram_tensor("out", [nt, P, K, dim], f16, kind="ExternalOutput").ap()

    n_const = 1 + int(with_b)

    with ExitStack() as ctx:
        e = ctx.enter_context
        xbuf = e(nc.sbuf_tensor([P, XB, K, 2, dim], f16))
        tmp = e(nc.sbuf_tensor([P, K, dim], f16))
        mbuf = e(nc.sbuf_tensor([P, 2, K, dim], f16))
        wrep = e(nc.sbuf_tensor([P, dim], f16))
        brep = e(nc.sbuf_tensor([P, dim], f16))
        s = e(nc.sbuf_tensor([P, 2, K], f32))  # ACT scale APs must be FP32
        const_sem = e(nc.semaphore("const_sem"))
        load_sems = [e(nc.semaphore(f"load_sem{j}")) for j in range(XB)]
        store_sems = [e(nc.semaphore(f"store_sem{j}")) for j in range(XB)]
        s_sem = e(nc.semaphore("s_sem"))
        act_sem = e(nc.semaphore("act_sem"))
        add_sem = e(nc.semaphore("add_sem"))
        chain_sem = e(nc.semaphore("chain_sem"))
        block = e(nc.Block())

        @block.sync
        def _(sync):
            sync.dma_start(out=wrep[:, :], in_=w_rep_d[:, :]).then_inc(const_sem, 16)
            if with_b:
                sync.dma_start(out=brep[:, :], in_=b_rep_d[:, :]).then_inc(
                    const_sem, 16
                )
            for t in range(nit):
                if t >= XB:
                    # slot free only after its previous store (o lives in the
                    # x_0 half of the slot) fully landed in DRAM
                    sync.wait_ge(store_sems[t % XB], 16 * (t // XB))
                sync.dma_start(
                    out=xbuf[:, t % XB, :, :, :], in_=xin[t % nt]
                ).then_inc(load_sems[t % XB], 16)

        def emit_b1(t):
            # batch1: tmp_k = x_l_k * w, s[:, t%2, k] = rowsum(tmp_k)
            sl = t % XB
            for k in range(K):
                nc.vector.scalar_tensor_tensor(
                    out=tmp[:, k, :],
                    in0=xbuf[:, sl, k, 0, :],
                    scalar=1.0,
                    in1=wrep[:, :],
                    op0=MUL,
                    op1=MUL,
                    accum_out=s[:, t % 2, k : k + 1],
                ).then_inc(s_sem, 1)

        def emit_adds(vector, t):
            # batch3: o(t) = m(t) + x_l(t) in H chunks, over the x_0 half
            sl = t % XB
            kc = K // H
            for h in range(H):
                k0, k1 = h * kc, (h + 1) * kc
                vector.wait_ge(act_sem, K * t + k1)
                inst = nc.vector.tensor_tensor(
                    out=xbuf[:, sl, k0:k1, 1, :],
                    in0=mbuf[:, t % 2, k0:k1, :],
                    in1=xbuf[:, sl, k0:k1, 0, :],
                    op=ADD,
                )
                if with_b:
                    inst.then_inc(chain_sem, 1)
                    vector.wait_ge(chain_sem, H * t + h + 1)
                    inst = nc.vector.tensor_tensor(
                        out=xbuf[:, sl, k0:k1, 1, :],
                        in0=xbuf[:, sl, k0:k1, 1, :],
                        in1=brep[:, None, :].broadcast_to([P, kc, dim]),
                        op=ADD,
                    )
                inst.then_inc(add_sem, 1)

        @block.vector
        def _(vector):
            vector.wait_ge(const_sem, 16 * n_const)
            for t in range(nit):
                vector.wait_ge(load_sems[t % XB], 16 * (t // XB + 1))
                emit_b1(t)
                if t >= 1:
                    emit_adds(vector, t - 1)
            emit_adds(vector, nit - 1)

        @block.scalar
        def _(scalar):
            def emit_acts(t):
                # batch2: m_k = Copy(x_0_k * s[:, t%2, k]), chained per-k
                sl = t % XB
                for k in range(K):
                    scalar.wait_ge(s_sem, K * t + k + 1)
                    nc.scalar.activation(
                        out=mbuf[:, t % 2, k, :],
                        in_=xbuf[:, sl, k, 1, :],
                        func=COPY,
                        scale=s[:, t % 2, k : k + 1],
                    ).then_inc(act_sem, 1)

            def emit_store(t):
                scalar.wait_ge(add_sem, H * (t + 1))
                scalar.dma_start(
                    out=out[t % nt], in_=xbuf[:, t % XB, :, 1, :]
                ).then_inc(store_sems[t % XB], 16)

            for t in range(nit):
                emit_acts(t)
                if t >= 1:
                    emit_store(t - 1)
            emit_store(nit - 1)
            # drain: all stores landed before program end
            for j in range(XB):
                n_j = (nit - 1 - j) // XB + 1 if j < nit else 0
                if n_j:
                    scalar.wait_ge(store_sems[j], 16 * n_j)

    return nc


_cache = {}


def _get_module(nb, dim, with_b, repeat=1):
    key = (nb, dim, with_b, repeat)
    if key not in _cache:
        _cache[key] = _build(nb, dim, with_b, repeat)
    return _cache[key]


def make_inputs(x_l, x_0, w, b, n_cores=N_CORES):
    """Host-side shard + tile-major interleave + fp16 cast."""
    rows, dim = x_l.shape
    assert rows % (n_cores * P) == 0
    bl = rows // n_cores
    nb = bl // P
    assert nb % K == 0
    nt = nb // K
    with_b = bool(np.any(b))
    # [rows, 2, dim] -> per core [nt, K, P, 2, dim] -> transpose to
    # [nt, P, K, 2, dim] so each (t, p) slab is contiguous (16 KB descriptors)
    xin = np.empty((rows, 2, dim), dtype=np.float16)
    xin[:, 0, :] = x_l
    xin[:, 1, :] = x_0
    w_rep = np.ascontiguousarray(
        np.broadcast_to(w.reshape(1, dim), (P, dim)).astype(np.float16)
    )
    if with_b:
        b_rep = np.ascontiguousarray(
            np.broadcast_to(b.reshape(1, dim), (P, dim)).astype(np.float16)
        )
    in_maps = []
    for i in range(n_cores):
        xc = xin[i * bl : (i + 1) * bl].reshape(nt, K, P, 2, dim)
        m = {
            "xin": np.ascontiguousarray(xc.transpose(0, 2, 1, 3, 4)),
            "w_rep_in": w_rep,
        }
        if with_b:
            m["b_rep_in"] = b_rep
        in_maps.append(m)
    return in_maps, with_b, nb, dim


def run_sharded(x_l, x_0, w, b, trace=False, repeat=1, **kw):
    in_maps, with_b, nb, dim = make_inputs(x_l, x_0, w, b)
    nc = _get_module(nb, dim, with_b, repeat=repeat)
    res = bass_utils.run_bass_kernel_spmd(
        nc, in_maps, core_ids=list(range(N_CORES)), trace=trace, **kw
    )
    # out is tile-major [nt, P, K, dim]; un-transpose back to [bl, dim]
    outs = []
    for i in range(N_CORES):
        o = res.results[i]["out"]
        nt = o.shape[0]
        outs.append(np.ascontiguousarray(o.transpose(0, 2, 1, 3)).reshape(-1, dim))
    out = np.concatenate(outs, axis=0)
    return out, res


def kernel(x_l, x_0, w, b):
    out, _ = run_sharded(
        np.asarray(x_l), np.asarray(x_0), np.asarray(w), np.asarray(b)
    )
    return out.astype(np.float32, copy=False)
